# revision 36
# baseline (speedup 1.0000x reference)
"""MoE kernel for TRN2, 8 NeuronCores, data-parallel over the batch dim.

Reference computation (B=8192, D=1024, H=1024, E=16):
    weights = softmax(x @ Wg + bg, axis=1)            # [B, E]
    h       = relu(einsum('bd,edh->beh', x, W1) + b1) # [B, E, H]
    eo      = einsum('beh,eh->be', h, W2) + b2        # [B, E]
    out     = sum(eo * weights, axis=1, keepdims=True)# [B, 1]

Strategy (v2 — bf16 matmuls, col-tiled stage 2, transposed combine):
  - Shard B over 8 cores (1024 rows/core); weights replicated.
  - All heavy matmuls in bf16 (1 cycle/row on PE, N=512 moving, fast
    weight load); contractions accumulate in fp32 PSUM, so the end-to-end
    error stays ~0.4% against the fp32 reference (gate is 2%).
  - Stage 1 per t=(e, h_tile): psum [h=128, b=512] x2 accumulated over 8
    d-tiles from resident xT tiles; ReLU+b1 via ScalarE -> hr bf16.
    Sustains the 216ns/matmul N=512 issue floor (~92% of runtime).
  - Stage 2: w2 column blocks, 4 PSUM col-groups (partitions 32j..32j+15,
    j=t%4): batches of 4 matmuls on distinct col-groups run concurrently
    in the PE array (measured 4x vs serial); flushed in 8-t batches one
    batch behind so the PE never waits on a fresh ReLU tile. Groups are
    summed by the replicated-weight reduction in the combine.
  - Gating stays transposed end-to-end: logits^T [128, B] with gate
    weights replicated into all 4 col-groups (pad cols zero); softmax is
    exp on ScalarE (bias=bg, pad rows -40 -> 0) + one PE reduction with a
    0.25-weighted ones vector (each expert appears 4x); no transposes.
    The gating + first stage-1 tile are interleaved per d-tile in DMA
    arrival order to fill the input-load window.
  - Combine: v = eo_psum * expw (DVE); num = ones^T @ v + (b2/4)^T @ expw;
    y = num * reciprocal(sumexp); y^T DMA'd out as a [1, B] row, pipelined
    per batch half across DVE/PE/DMA.
  - PSUM budget is exactly 8 banks: 4x ps1 + 2x eo + 2x aux (glog, sumexp
    and num share one rotating slot; their lifetimes are disjoint).
"""

import numpy as np
import ml_dtypes

import concourse.bacc as bacc
import concourse.bass as bass
import concourse.mybir as mybir
from concourse import tile
from concourse.bass_utils import run_bass_kernel_spmd

B, D, H, E = 8192, 1024, 1024, 16
N_CORES = 8
BS = B // N_CORES  # 1024 batch rows per core
BH = 512           # psum-bank-sized half of the batch
DT = D // 128      # 8 d-tiles
HT = H // 128      # 8 h-tiles
T = E * HT         # 128 (e, h_tile) pairs
GB = 4             # stage-2 col-groups

F32 = mybir.dt.float32
F32R = mybir.dt.float32r
BF16 = mybir.dt.bfloat16
AF = mybir.ActivationFunctionType


def build_bass():
    nc = bacc.Bacc("TRN2", target_bir_lowering=False, debug=False)
    xt_d = nc.dram_tensor("xt", [D, BS], BF16, kind="ExternalInput")
    w1_d = nc.dram_tensor("w1p", [T, 128, DT * 128], BF16, kind="ExternalInput")
    b1t_d = nc.dram_tensor("b1t", [128, T], F32, kind="ExternalInput")
    w2bd_d = nc.dram_tensor("w2bd", [128, T * E], BF16, kind="ExternalInput")
    wg4_d = nc.dram_tensor("wg4", [128, DT * 128], BF16, kind="ExternalInput")
    bg4_d = nc.dram_tensor("bg4", [128, 1], F32, kind="ExternalInput")
    b2q4_d = nc.dram_tensor("b2q4", [128, 1], BF16, kind="ExternalInput")
    ones1_d = nc.dram_tensor("ones1", [128, 1], BF16, kind="ExternalInput")
    o025_d = nc.dram_tensor("o025", [128, 1], BF16, kind="ExternalInput")
    y_d = nc.dram_tensor("y", [1, BS], F32, kind="ExternalOutput")

    with tile.TileContext(nc) as tc:
        with (
            tc.tile_pool(name="const", bufs=1) as cpool,
            tc.tile_pool(name="w1", bufs=6) as w1pool,
            tc.tile_pool(name="hrelu", bufs=18) as hpool,
            tc.tile_pool(name="misc", bufs=1) as mpool,
            tc.tile_pool(name="ps1", bufs=4, space=bass.MemorySpace.PSUM) as psh,
            tc.tile_pool(name="ps_eo", bufs=1, space=bass.MemorySpace.PSUM) as pseo,
            tc.tile_pool(name="ps_aux", bufs=1, space=bass.MemorySpace.PSUM) as psaux,
        ):
            # ---- resident tensors ----
            # DMAs are queued per issuing engine (sync + scalar are both
            # hardware queues; gpsimd's is the slow software path — avoid).
            # Even xt tiles ride sync; scalar carries the gate weights, the
            # first W1 tile (needed by the prologue), then the odd xt tiles,
            # then the W1 stream. The prologue consumes d-tiles in this
            # arrival order.
            wg4_sb = cpool.tile([128, DT * 128], BF16, tag="wg4")
            nc.scalar.dma_start(wg4_sb[:], wg4_d[:])
            xt_sb = [None] * DT
            for d in range(0, DT, 2):
                tl = cpool.tile([128, BS], BF16, tag=f"xt{d}")
                nc.sync.dma_start(tl[:], xt_d[d * 128:(d + 1) * 128, :])
                xt_sb[d] = tl
            w1t01 = []
            for t in range(1):
                w1t = w1pool.tile([128, DT * 128], BF16, tag="w1t")
                nc.scalar.dma_start(w1t[:], w1_d[t, :, :])
                w1t01.append(w1t)
            for d in range(1, DT, 2):
                tl = cpool.tile([128, BS], BF16, tag=f"xt{d}")
                nc.scalar.dma_start(tl[:], xt_d[d * 128:(d + 1) * 128, :])
                xt_sb[d] = tl
            bg4_sb = cpool.tile([128, 1], F32, tag="bg4")
            nc.sync.dma_start(bg4_sb[:], bg4_d[:])
            b1t_sb = cpool.tile([128, T], F32, tag="b1t")
            nc.sync.dma_start(b1t_sb[:], b1t_d[:])
            b2q4_sb = cpool.tile([128, 1], BF16, tag="b2q4")
            nc.sync.dma_start(b2q4_sb[:], b2q4_d[:])
            ones1_sb = cpool.tile([128, 1], BF16, tag="ones1")
            nc.sync.dma_start(ones1_sb[:], ones1_d[:])
            o025_sb = cpool.tile([128, 1], BF16, tag="o025")
            nc.sync.dma_start(o025_sb[:], o025_d[:])
            w2bd_sb = cpool.tile([128, T * E], BF16, tag="w2bd")

            expw_sb = mpool.tile([128, BS], BF16, tag="expw")
            v_sb = mpool.tile([128, BS], BF16, tag="v")
            serec_sb = mpool.tile([1, BS], F32, tag="serec")
            y_sb = mpool.tile([1, BS], F32, tag="ysb")

            # ---- stage-2 accumulators (one per batch half, so a DVE read
            # of one half never serializes matmul writes to the other);
            # zero the pad rows once ----
            eo_hs = []
            for bh in range(2):
                eo_h = pseo.tile([128, BH], F32, tag=f"eo{bh}", name=f"eo{bh}")
                nc.vector.memset(eo_h[:], 0.0)
                eo_hs.append(eo_h)

            hrs = {}

            # ---- prologue: gating + t=0 interleaved per d-tile ----
            # Both accumulations consume xt[d] the moment it lands, in DMA
            # arrival order (even tiles first), so the PE stream starts
            # while the input is still loading.
            glog = psaux.tile([128, BS], F32, tag="aux")
            ps01 = [[psh.tile([128, BH], F32, tag="ps1", name=f"ps1p{t}{bh}")
                     for bh in range(2)] for t in range(1)]
            ARRIVAL = list(range(0, DT, 2)) + list(range(1, DT, 2))
            for k, d in enumerate(ARRIVAL):
                first, last = (k == 0), (k == DT - 1)
                for bh in range(2):
                    nc.tensor.matmul(
                        glog[:, bh * BH:(bh + 1) * BH],
                        wg4_sb[:, d * 128:(d + 1) * 128],
                        xt_sb[d][:, bh * BH:(bh + 1) * BH],
                        start=first, stop=last,
                        skip_group_check=True,
                    )
                    for t in range(1):
                        nc.tensor.matmul(
                            ps01[t][bh][:],
                            w1t01[t][:, d * 128:(d + 1) * 128],
                            xt_sb[d][:, bh * BH:(bh + 1) * BH],
                            start=first, stop=last,
                            skip_group_check=True,
                        )
            # expw = exp(logits + bg); pad rows get bias -40 -> ~0
            nc.scalar.activation(expw_sb[:], glog[:], AF.Exp, bias=bg4_sb[:])
            for t in range(1):
                hr = hpool.tile([128, BS], BF16, tag="hr")
                for bh in range(2):
                    nc.scalar.activation(
                        hr[:, bh * BH:(bh + 1) * BH], ps01[t][bh][:], AF.Relu,
                        bias=b1t_sb[:, t:t + 1],
                    )
                hrs[t] = hr

            def flush(ts, after_bh=None):
                for bh in range(2):
                    for tt in ts:
                        g = tt % GB
                        nc.tensor.matmul(
                            eo_hs[bh][32 * g:32 * g + 16, :],
                            w2bd_sb[:, tt * E:(tt + 1) * E],
                            hrs[tt][:, bh * BH:(bh + 1) * BH],
                            start=(tt < GB), stop=(tt >= T - GB),
                            skip_group_check=True,
                            tile_position=(0, 32 * g),
                        )
                    if after_bh is not None:
                        after_bh(bh)
                for tt in ts:
                    del hrs[tt]

            # ---- main loop over t=(e, h_tile) ----
            # Stage-2 is flushed in 8-t batches, one full batch behind, so
            # the PE never waits on a freshly produced ReLU tile and the
            # full-array <-> col-tiled pipeline bubble is paid 16x, not 32x.
            FB = 2 * GB
            for t in range(1, T):
                if t % FB == 0 and t >= 2 * FB:
                    flush(range(t - 2 * FB, t - FB))
                w1t = w1pool.tile([128, DT * 128], BF16, tag="w1t")
                nc.scalar.dma_start(w1t[:], w1_d[t, :, :])
                if t == 2:
                    nc.scalar.dma_start(w2bd_sb[:], w2bd_d[:])
                hr = hpool.tile([128, BS], BF16, tag="hr")
                for bh in range(2):
                    ps1 = psh.tile([128, BH], F32, tag="ps1")
                    for d in range(DT):
                        nc.tensor.matmul(
                            ps1[:],
                            w1t[:, d * 128:(d + 1) * 128],
                            xt_sb[d][:, bh * BH:(bh + 1) * BH],
                            start=(d == 0), stop=(d == DT - 1),
                            skip_group_check=True,
                        )
                    nc.scalar.activation(
                        hr[:, bh * BH:(bh + 1) * BH], ps1[:], AF.Relu,
                        bias=b1t_sb[:, t:t + 1],
                    )
                hrs[t] = hr
                if t == 2:
                    # sum of gate weights (each expert appears 4x -> 0.25)
                    sumexp = psaux.tile([1, BS], F32, tag="aux")
                    for bh in range(2):
                        nc.tensor.matmul(
                            sumexp[:, bh * BH:(bh + 1) * BH],
                            o025_sb[:], expw_sb[:, bh * BH:(bh + 1) * BH],
                            start=True, stop=True, skip_group_check=True,
                        )
                    nc.vector.reciprocal(serec_sb[:], sumexp[:])
            # ---- last two stage-2 batches + combine, pipelined ----
            # All bh0 stage-2 matmuls first (closing the bh0 psum groups),
            # then v0 on DVE overlaps the bh1 matmuls; num reductions and
            # the y multiply/DMA pipeline per half behind that.
            halves = [slice(0, BH), slice(BH, BS)]
            for bh, sl in enumerate(halves):
                for tt in range(T - 2 * FB, T):
                    g = tt % GB
                    nc.tensor.matmul(
                        eo_hs[bh][32 * g:32 * g + 16, :],
                        w2bd_sb[:, tt * E:(tt + 1) * E],
                        hrs[tt][:, sl],
                        start=False, stop=(tt >= T - GB),
                        skip_group_check=True,
                        tile_position=(0, 32 * g),
                    )
                nc.vector.tensor_mul(v_sb[:, sl], eo_hs[bh][:], expw_sb[:, sl])
            for tt in range(T - 2 * FB, T):
                del hrs[tt]

            # ---- combine: y = (1^T(eo*expw) + (b2/4)^T expw) / sumexp ----
            num = psaux.tile([1, BS], F32, tag="aux")
            for sl in halves:
                nc.tensor.matmul(
                    num[:, sl], ones1_sb[:], v_sb[:, sl],
                    start=True, stop=False, skip_group_check=True,
                )
                nc.tensor.matmul(
                    num[:, sl], b2q4_sb[:], expw_sb[:, sl],
                    start=False, stop=True, skip_group_check=True,
                )
                nc.vector.tensor_mul(y_sb[:, sl], num[:, sl], serec_sb[:, sl])
                nc.sync.dma_start(y_d[:, sl], y_sb[:, sl])
    nc.compile()
    return nc


def prep_inputs(x, W1, b1, W2, b2, Wg, bg):
    """Host-side data prep. Returns (shared_map, per_core_xt)."""
    f = np.float32
    bf = ml_dtypes.bfloat16
    # W1 [E, D, H] -> [t=(e,ht), d_in, (d_t, h_in)]: per t one contiguous
    # block whose SBUF layout is [128 d_in, 8 d_t * 128 h]
    w1p = np.ascontiguousarray(
        np.asarray(W1, f).reshape(E, DT, 128, HT, 128)
        .transpose(0, 3, 2, 1, 4).reshape(T, 128, DT * 128)).astype(bf)
    b1t = np.ascontiguousarray(
        np.asarray(b1, f).reshape(E, HT, 128).transpose(2, 0, 1).reshape(128, T))
    w2bd = np.zeros((128, T, E), dtype=f)
    for t in range(T):
        e, ht = divmod(t, HT)
        w2bd[:, t, e] = W2[e, ht * 128:(ht + 1) * 128]
    w2bd = w2bd.reshape(128, T * E).astype(bf)
    # gate weights replicated into the 4 col-groups (16 used + 16 pad cols)
    wgr = np.asarray(Wg, f).reshape(DT, 128, E)
    wg4 = np.zeros((DT, 128, 128), dtype=f)
    for j in range(GB):
        wg4[:, :, 32 * j:32 * j + E] = wgr
    wg4 = np.ascontiguousarray(wg4.transpose(1, 0, 2).reshape(128, DT * 128)).astype(bf)
    lane = np.arange(128) % 32
    real = lane < E
    bg4 = np.full((128, 1), -40.0, f)
    bg4[real, 0] = np.tile(np.asarray(bg, f), GB)
    b2q4 = np.zeros((128, 1), f)
    b2q4[real, 0] = np.tile(np.asarray(b2, f) / 4.0, GB)
    b2q4 = b2q4.astype(bf)
    ones1 = np.where(real, 1.0, 0.0).astype(bf).reshape(128, 1)
    o025 = np.where(real, 0.25, 0.0).astype(bf).reshape(128, 1)
    shared = {"w1p": w1p, "b1t": b1t, "w2bd": w2bd, "wg4": wg4, "bg4": bg4,
              "b2q4": b2q4, "ones1": ones1, "o025": o025}
    xT = np.asarray(x, f).T.astype(bf)  # [D, B]
    xts = [np.ascontiguousarray(xT[:, c * BS:(c + 1) * BS]) for c in range(N_CORES)]
    return shared, xts


def run(inputs, trace=False):
    nc = build_bass()
    shared, xts = prep_inputs(**inputs)
    in_maps = [dict(shared, xt=xts[c]) for c in range(N_CORES)]
    res = run_bass_kernel_spmd(
        nc, in_maps, core_ids=list(range(N_CORES)), trace=trace
    )
    y = np.concatenate([r["y"] for r in res.results], axis=1)  # [1, B]
    return np.ascontiguousarray(y.reshape(B, 1).astype(np.float32)), res


def kernel(**inputs):
    y, _ = run(inputs, trace=False)
    return y


if __name__ == "__main__":
    rng = np.random.default_rng(0)
    ins = {
        "x": rng.standard_normal((B, D), dtype=np.float32),
        "W1": rng.standard_normal((E, D, H), dtype=np.float32) / 32,
        "b1": rng.standard_normal((E, H), dtype=np.float32) / 32,
        "W2": rng.standard_normal((E, H), dtype=np.float32) / 32,
        "b2": rng.standard_normal((E,), dtype=np.float32) / 32,
        "Wg": rng.standard_normal((D, E), dtype=np.float32) / 32,
        "bg": rng.standard_normal((E,), dtype=np.float32) / 32,
    }
    y = kernel(**ins)
    print("ok", y.shape, y.dtype)


# revision 37
# speedup vs baseline: 1.0091x; 1.0091x over previous
"""MoE kernel for TRN2, 8 NeuronCores, data-parallel over the batch dim.

Reference computation (B=8192, D=1024, H=1024, E=16):
    weights = softmax(x @ Wg + bg, axis=1)            # [B, E]
    h       = relu(einsum('bd,edh->beh', x, W1) + b1) # [B, E, H]
    eo      = einsum('beh,eh->be', h, W2) + b2        # [B, E]
    out     = sum(eo * weights, axis=1, keepdims=True)# [B, 1]

Strategy (v2 — bf16 matmuls, col-tiled stage 2, transposed combine):
  - Shard B over 8 cores (1024 rows/core); weights replicated.
  - All heavy matmuls in bf16 (1 cycle/row on PE, N=512 moving, fast
    weight load); contractions accumulate in fp32 PSUM, so the end-to-end
    error stays ~0.4% against the fp32 reference (gate is 2%).
  - Stage 1 per t=(e, h_tile): psum [h=128, b=512] x2 accumulated over 8
    d-tiles from resident xT tiles; ReLU+b1 via ScalarE -> hr bf16.
    Sustains the 216ns/matmul N=512 issue floor (~92% of runtime).
  - Stage 2: w2 column blocks, 4 PSUM col-groups (partitions 32j..32j+15,
    j=t%4): batches of 4 matmuls on distinct col-groups run concurrently
    in the PE array (measured 4x vs serial); flushed in 8-t batches one
    batch behind so the PE never waits on a fresh ReLU tile. Groups are
    summed by the replicated-weight reduction in the combine.
  - Gating stays transposed end-to-end: logits^T [128, B] with gate
    weights replicated into all 4 col-groups (pad cols zero); softmax is
    exp on ScalarE (bias=bg, pad rows -40 -> 0) + one PE reduction with a
    0.25-weighted ones vector (each expert appears 4x); no transposes.
    The gating + first stage-1 tile are interleaved per d-tile in DMA
    arrival order to fill the input-load window.
  - Combine: v = eo_psum * expw (DVE); num = ones^T @ v + (b2/4)^T @ expw;
    y = num * reciprocal(sumexp); y^T DMA'd out as a [1, B] row, pipelined
    per batch half across DVE/PE/DMA.
  - PSUM budget is exactly 8 banks: 4x ps1 + 2x eo + 2x aux (glog, sumexp
    and num share one rotating slot; their lifetimes are disjoint).
"""

import numpy as np
import ml_dtypes

import concourse.bacc as bacc
import concourse.bass as bass
import concourse.mybir as mybir
from concourse import tile
from concourse.bass_utils import run_bass_kernel_spmd

B, D, H, E = 8192, 1024, 1024, 16
N_CORES = 8
BS = B // N_CORES  # 1024 batch rows per core
BH = 512           # psum-bank-sized half of the batch
DT = D // 128      # 8 d-tiles
HT = H // 128      # 8 h-tiles
T = E * HT         # 128 (e, h_tile) pairs
GB = 4             # stage-2 col-groups

F32 = mybir.dt.float32
F32R = mybir.dt.float32r
BF16 = mybir.dt.bfloat16
AF = mybir.ActivationFunctionType


def build_bass():
    nc = bacc.Bacc("TRN2", target_bir_lowering=False, debug=False)
    xt_d = nc.dram_tensor("xt", [D, BS], BF16, kind="ExternalInput")
    w1_d = nc.dram_tensor("w1p", [T // 2, 128, 2 * DT * 128], BF16, kind="ExternalInput")
    b1t_d = nc.dram_tensor("b1t", [128, T], F32, kind="ExternalInput")
    w2bd_d = nc.dram_tensor("w2bd", [128, T * E], BF16, kind="ExternalInput")
    wg4_d = nc.dram_tensor("wg4", [128, DT * 128], BF16, kind="ExternalInput")
    bg4_d = nc.dram_tensor("bg4", [128, 1], F32, kind="ExternalInput")
    b2q4_d = nc.dram_tensor("b2q4", [128, 1], BF16, kind="ExternalInput")
    ones1_d = nc.dram_tensor("ones1", [128, 1], BF16, kind="ExternalInput")
    o025_d = nc.dram_tensor("o025", [128, 1], BF16, kind="ExternalInput")
    y_d = nc.dram_tensor("y", [1, BS], F32, kind="ExternalOutput")

    with tile.TileContext(nc) as tc:
        with (
            tc.tile_pool(name="const", bufs=1) as cpool,
            tc.tile_pool(name="w1", bufs=3) as w1pool,
            tc.tile_pool(name="hrelu", bufs=18) as hpool,
            tc.tile_pool(name="misc", bufs=1) as mpool,
            tc.tile_pool(name="ps1", bufs=4, space=bass.MemorySpace.PSUM) as psh,
            tc.tile_pool(name="ps_eo", bufs=1, space=bass.MemorySpace.PSUM) as pseo,
            tc.tile_pool(name="ps_aux", bufs=1, space=bass.MemorySpace.PSUM) as psaux,
        ):
            # ---- resident tensors ----
            # DMAs are queued per issuing engine (sync + scalar are both
            # hardware queues; gpsimd's is the slow software path — avoid).
            # Even xt tiles ride sync; scalar carries the gate weights, the
            # first W1 tile (needed by the prologue), then the odd xt tiles,
            # then the W1 stream. The prologue consumes d-tiles in this
            # arrival order.
            wg4_sb = cpool.tile([128, DT * 128], BF16, tag="wg4")
            nc.scalar.dma_start(wg4_sb[:], wg4_d[:])
            xt_sb = [None] * DT
            for d in range(0, DT, 2):
                tl = cpool.tile([128, BS], BF16, tag=f"xt{d}")
                nc.sync.dma_start(tl[:], xt_d[d * 128:(d + 1) * 128, :])
                xt_sb[d] = tl
            w1pair = w1pool.tile([128, 2 * DT * 128], BF16, tag="w1t")
            nc.scalar.dma_start(w1pair[:], w1_d[0, :, :])
            for d in range(1, DT, 2):
                tl = cpool.tile([128, BS], BF16, tag=f"xt{d}")
                nc.scalar.dma_start(tl[:], xt_d[d * 128:(d + 1) * 128, :])
                xt_sb[d] = tl
            bg4_sb = cpool.tile([128, 1], F32, tag="bg4")
            nc.sync.dma_start(bg4_sb[:], bg4_d[:])
            b1t_sb = cpool.tile([128, T], F32, tag="b1t")
            nc.sync.dma_start(b1t_sb[:], b1t_d[:])
            b2q4_sb = cpool.tile([128, 1], BF16, tag="b2q4")
            nc.sync.dma_start(b2q4_sb[:], b2q4_d[:])
            ones1_sb = cpool.tile([128, 1], BF16, tag="ones1")
            nc.sync.dma_start(ones1_sb[:], ones1_d[:])
            o025_sb = cpool.tile([128, 1], BF16, tag="o025")
            nc.sync.dma_start(o025_sb[:], o025_d[:])
            w2bd_sb = cpool.tile([128, T * E], BF16, tag="w2bd")

            expw_sb = mpool.tile([128, BS], BF16, tag="expw")
            v_sb = mpool.tile([128, BS], BF16, tag="v")
            serec_sb = mpool.tile([1, BS], F32, tag="serec")
            y_sb = mpool.tile([1, BS], F32, tag="ysb")

            # ---- stage-2 accumulators (one per batch half, so a DVE read
            # of one half never serializes matmul writes to the other);
            # zero the pad rows once ----
            eo_hs = []
            for bh in range(2):
                eo_h = pseo.tile([128, BH], F32, tag=f"eo{bh}", name=f"eo{bh}")
                nc.vector.memset(eo_h[:], 0.0)
                eo_hs.append(eo_h)

            hrs = {}

            # ---- prologue: gating + t=0 interleaved per d-tile ----
            # Both accumulations consume xt[d] the moment it lands, in DMA
            # arrival order (even tiles first), so the PE stream starts
            # while the input is still loading.
            glog = psaux.tile([128, BS], F32, tag="aux")
            ps01 = [[psh.tile([128, BH], F32, tag="ps1", name=f"ps1p{t}{bh}")
                     for bh in range(2)] for t in range(1)]
            ARRIVAL = list(range(0, DT, 2)) + list(range(1, DT, 2))
            for k, d in enumerate(ARRIVAL):
                first, last = (k == 0), (k == DT - 1)
                for bh in range(2):
                    nc.tensor.matmul(
                        glog[:, bh * BH:(bh + 1) * BH],
                        wg4_sb[:, d * 128:(d + 1) * 128],
                        xt_sb[d][:, bh * BH:(bh + 1) * BH],
                        start=first, stop=last,
                        skip_group_check=True,
                    )
                    for t in range(1):
                        nc.tensor.matmul(
                            ps01[t][bh][:],
                            w1pair[:, d * 128:(d + 1) * 128],
                            xt_sb[d][:, bh * BH:(bh + 1) * BH],
                            start=first, stop=last,
                            skip_group_check=True,
                        )
            # expw = exp(logits + bg); pad rows get bias -40 -> ~0
            nc.scalar.activation(expw_sb[:], glog[:], AF.Exp, bias=bg4_sb[:])
            for t in range(1):
                hr = hpool.tile([128, BS], BF16, tag="hr")
                for bh in range(2):
                    nc.scalar.activation(
                        hr[:, bh * BH:(bh + 1) * BH], ps01[t][bh][:], AF.Relu,
                        bias=b1t_sb[:, t:t + 1],
                    )
                hrs[t] = hr

            def flush(ts, after_bh=None):
                for bh in range(2):
                    for tt in ts:
                        g = tt % GB
                        nc.tensor.matmul(
                            eo_hs[bh][32 * g:32 * g + 16, :],
                            w2bd_sb[:, tt * E:(tt + 1) * E],
                            hrs[tt][:, bh * BH:(bh + 1) * BH],
                            start=(tt < GB), stop=(tt >= T - GB),
                            skip_group_check=True,
                            tile_position=(0, 32 * g),
                        )
                    if after_bh is not None:
                        after_bh(bh)
                for tt in ts:
                    del hrs[tt]

            # ---- main loop over t=(e, h_tile) ----
            # Stage-2 is flushed in 8-t batches, one full batch behind, so
            # the PE never waits on a freshly produced ReLU tile and the
            # full-array <-> col-tiled pipeline bubble is paid 16x, not 32x.
            FB = 2 * GB
            for t in range(1, T):
                if t % FB == 0 and t >= 2 * FB:
                    flush(range(t - 2 * FB, t - FB))
                if t % 2 == 0:
                    w1pair = w1pool.tile([128, 2 * DT * 128], BF16, tag="w1t")
                    nc.scalar.dma_start(w1pair[:], w1_d[t // 2, :, :])
                if t == 2:
                    nc.scalar.dma_start(w2bd_sb[:], w2bd_d[:])
                j = t % 2
                hr = hpool.tile([128, BS], BF16, tag="hr")
                for bh in range(2):
                    ps1 = psh.tile([128, BH], F32, tag="ps1")
                    for d in range(DT):
                        nc.tensor.matmul(
                            ps1[:],
                            w1pair[:, (j * DT + d) * 128:(j * DT + d + 1) * 128],
                            xt_sb[d][:, bh * BH:(bh + 1) * BH],
                            start=(d == 0), stop=(d == DT - 1),
                            skip_group_check=True,
                        )
                    nc.scalar.activation(
                        hr[:, bh * BH:(bh + 1) * BH], ps1[:], AF.Relu,
                        bias=b1t_sb[:, t:t + 1],
                    )
                hrs[t] = hr
                if t == 2:
                    # sum of gate weights (each expert appears 4x -> 0.25)
                    sumexp = psaux.tile([1, BS], F32, tag="aux")
                    for bh in range(2):
                        nc.tensor.matmul(
                            sumexp[:, bh * BH:(bh + 1) * BH],
                            o025_sb[:], expw_sb[:, bh * BH:(bh + 1) * BH],
                            start=True, stop=True, skip_group_check=True,
                        )
                    nc.vector.reciprocal(serec_sb[:], sumexp[:])
            # ---- last two stage-2 batches + combine, pipelined ----
            # All bh0 stage-2 matmuls first (closing the bh0 psum groups),
            # then v0 on DVE overlaps the bh1 matmuls; num reductions and
            # the y multiply/DMA pipeline per half behind that.
            halves = [slice(0, BH), slice(BH, BS)]
            for bh, sl in enumerate(halves):
                for tt in range(T - 2 * FB, T):
                    g = tt % GB
                    nc.tensor.matmul(
                        eo_hs[bh][32 * g:32 * g + 16, :],
                        w2bd_sb[:, tt * E:(tt + 1) * E],
                        hrs[tt][:, sl],
                        start=False, stop=(tt >= T - GB),
                        skip_group_check=True,
                        tile_position=(0, 32 * g),
                    )
                nc.vector.tensor_mul(v_sb[:, sl], eo_hs[bh][:], expw_sb[:, sl])
            for tt in range(T - 2 * FB, T):
                del hrs[tt]

            # ---- combine: y = (1^T(eo*expw) + (b2/4)^T expw) / sumexp ----
            num = psaux.tile([1, BS], F32, tag="aux")
            for sl in halves:
                nc.tensor.matmul(
                    num[:, sl], ones1_sb[:], v_sb[:, sl],
                    start=True, stop=False, skip_group_check=True,
                )
                nc.tensor.matmul(
                    num[:, sl], b2q4_sb[:], expw_sb[:, sl],
                    start=False, stop=True, skip_group_check=True,
                )
                nc.vector.tensor_mul(y_sb[:, sl], num[:, sl], serec_sb[:, sl])
                nc.sync.dma_start(y_d[:, sl], y_sb[:, sl])
    nc.compile()
    return nc


def prep_inputs(x, W1, b1, W2, b2, Wg, bg):
    """Host-side data prep. Returns (shared_map, per_core_xt)."""
    f = np.float32
    bf = ml_dtypes.bfloat16
    # W1 [E, D, H] -> [t=(e,ht), d_in, (d_t, h_in)]: per t one contiguous
    # block whose SBUF layout is [128 d_in, 8 d_t * 128 h]
    w1p = np.ascontiguousarray(
        np.asarray(W1, f).reshape(E, DT, 128, HT, 128)
        .transpose(0, 3, 2, 1, 4).reshape(T, 128, DT * 128))
    w1p = np.ascontiguousarray(
        w1p.reshape(T // 2, 2, 128, DT * 128).transpose(0, 2, 1, 3)
        .reshape(T // 2, 128, 2 * DT * 128)).astype(bf)
    b1t = np.ascontiguousarray(
        np.asarray(b1, f).reshape(E, HT, 128).transpose(2, 0, 1).reshape(128, T))
    w2bd = np.zeros((128, T, E), dtype=f)
    for t in range(T):
        e, ht = divmod(t, HT)
        w2bd[:, t, e] = W2[e, ht * 128:(ht + 1) * 128]
    w2bd = w2bd.reshape(128, T * E).astype(bf)
    # gate weights replicated into the 4 col-groups (16 used + 16 pad cols)
    wgr = np.asarray(Wg, f).reshape(DT, 128, E)
    wg4 = np.zeros((DT, 128, 128), dtype=f)
    for j in range(GB):
        wg4[:, :, 32 * j:32 * j + E] = wgr
    wg4 = np.ascontiguousarray(wg4.transpose(1, 0, 2).reshape(128, DT * 128)).astype(bf)
    lane = np.arange(128) % 32
    real = lane < E
    bg4 = np.full((128, 1), -40.0, f)
    bg4[real, 0] = np.tile(np.asarray(bg, f), GB)
    b2q4 = np.zeros((128, 1), f)
    b2q4[real, 0] = np.tile(np.asarray(b2, f) / 4.0, GB)
    b2q4 = b2q4.astype(bf)
    ones1 = np.where(real, 1.0, 0.0).astype(bf).reshape(128, 1)
    o025 = np.where(real, 0.25, 0.0).astype(bf).reshape(128, 1)
    shared = {"w1p": w1p, "b1t": b1t, "w2bd": w2bd, "wg4": wg4, "bg4": bg4,
              "b2q4": b2q4, "ones1": ones1, "o025": o025}
    xT = np.asarray(x, f).T.astype(bf)  # [D, B]
    xts = [np.ascontiguousarray(xT[:, c * BS:(c + 1) * BS]) for c in range(N_CORES)]
    return shared, xts


def run(inputs, trace=False):
    nc = build_bass()
    shared, xts = prep_inputs(**inputs)
    in_maps = [dict(shared, xt=xts[c]) for c in range(N_CORES)]
    res = run_bass_kernel_spmd(
        nc, in_maps, core_ids=list(range(N_CORES)), trace=trace
    )
    y = np.concatenate([r["y"] for r in res.results], axis=1)  # [1, B]
    return np.ascontiguousarray(y.reshape(B, 1).astype(np.float32)), res


def kernel(**inputs):
    y, _ = run(inputs, trace=False)
    return y


if __name__ == "__main__":
    rng = np.random.default_rng(0)
    ins = {
        "x": rng.standard_normal((B, D), dtype=np.float32),
        "W1": rng.standard_normal((E, D, H), dtype=np.float32) / 32,
        "b1": rng.standard_normal((E, H), dtype=np.float32) / 32,
        "W2": rng.standard_normal((E, H), dtype=np.float32) / 32,
        "b2": rng.standard_normal((E,), dtype=np.float32) / 32,
        "Wg": rng.standard_normal((D, E), dtype=np.float32) / 32,
        "bg": rng.standard_normal((E,), dtype=np.float32) / 32,
    }
    y = kernel(**ins)
    print("ok", y.shape, y.dtype)
